# revision 27
# baseline (speedup 1.0000x reference)
"""Trainium2 Bass kernel for nn_DialogActLabeller (segment_reduce).

Computes, for input enc_output [32, 4096, 1024], W [1024, 256], b [256],
cls_pos [32, 64], last_sep [32]:

    x = enc_output @ W + b                      # [B, S, 256]
    seg[b, n] = sum_{s in [start_n, end_n)} x[b, s, :]
    out = log_softmax(seg, axis=-1)             # [B, 64, 256]

Key algebraic restructure: the projection is linear, so segment-reduce
FIRST on enc_output (via a matmul with a 0/1 segment-indicator matrix A),
then project the tiny [64, 1024] per-batch result with W, and add
len_n * b for the bias.  This reads enc_output exactly once from HBM and
does ~1/32 of the naive FLOPs.

The kernel is HBM-bound (enc_output is 512 MiB), so enc is shipped as
fp8 e4m3 with error diffusion along s so segment sums telescope (see
_quantize_diffuse).  The segment-reduce matmul runs in fp8 DoubleRow
mode; in the warm (K=8/8) HAM state the PE issues one F=512 DR matmul
every 213 ns with the weight loads fully overlapped, which keeps pace
with the 16-engine enc DMA stream at ~358 GB/s.

PE clock management: the PE_HAM clock gate defaults to K=4/8 (1.2 GHz)
and only reaches 2.4 GHz after ~3.4 us of gap-free busy.  A warm-up
burst of dummy DR matmuls runs during the DMA ramp so the array is
already warm when the first enc slab lands, and small filler matmuls
are sprinkled between slabs so no idle window re-throttles the clock.

The per-batch tail (PSUM evict, transpose, projection, softmax) runs in
fp16 (fast transposes + FWL-overlapped weight loads, ample precision
against the 2e-2 gate), is software-pipelined one batch behind the enc
stream, and the softmax chain is fused:  exp(x-max) with the max as the
ACT bias input and the sum as ACT accum_out, then a single
(sv + negmax) - lse vector op.  Both ACT tables (Copy/Exp, Ln) are
preloaded at kernel start so no table load lands on the critical path.
Outputs DMA per batch as soon as they are ready.

Sharding: pure data parallel, 4 batch rows per core across 8 cores
(W, b replicated), no cross-core communication.
"""

import numpy as np

import concourse.bacc as bacc
import concourse.bass as bass
import concourse.tile as tile
from concourse import mybir
from concourse import bass_utils
from contextlib import ExitStack

# Problem shapes (hardcoded per contract)
B, S, D_IN, D_OUT, N_SENT = 32, 4096, 1024, 256, 64
N_CORES = 8
BPC = B // N_CORES          # batches per core
SCHUNKS = S // 128          # 32 sequence chunks of 128
DCH = D_IN // 128           # 8 d_in chunks of 128
SS_PER_DMA = 8              # s-chunks per enc DMA (1 MiB fp8 transfers)
N_DMA = SCHUNKS // SS_PER_DMA

F32 = mybir.dt.float32
F16 = mybir.dt.float16
FP8 = mybir.dt.float8e4
_E4NP = mybir.dt.np(FP8)    # ml_dtypes.float8_e4m3
_F16NP = mybir.dt.np(F16)

WARMUP_MMS = 12             # dummy DR matmuls to un-throttle the PE HAM
FILLERS_PER_SLAB = 1        # keep-warm matmuls after each slab's real MMs


def _build_program():
    nc = bacc.Bacc("TRN2", debug=False)

    enc = nc.dram_tensor(
        "enc", [BPC, N_DMA, 128, SS_PER_DMA * D_IN], FP8, kind="ExternalInput"
    ).ap()
    # W host-pre-tiled to fp16 [128, j, o] with d = j*128 + p
    wt = nc.dram_tensor("w", [128, DCH * D_OUT], F16, kind="ExternalInput").ap()
    bias = nc.dram_tensor("bias", [D_OUT], F32, kind="ExternalInput").ap()
    # the segment-indicator matrix A is generated ON DEVICE from these tiny
    # inputs (s-grid + per-batch start/end vectors): streaming the 1 MiB A
    # matrix from HBM would add ~3 us to the critical DMA-engine finish
    sgrid = nc.dram_tensor("sgrid", [128, SCHUNKS], F32, kind="ExternalInput").ap()
    sten = nc.dram_tensor("sten", [BPC, 2 * N_SENT], F32, kind="ExternalInput").ap()
    lens = nc.dram_tensor("lens", [N_SENT, BPC], F32, kind="ExternalInput").ap()
    ident = nc.dram_tensor("ident", [N_SENT, N_SENT], F16, kind="ExternalInput").ap()
    out = nc.dram_tensor(
        "out", [BPC, N_SENT, D_OUT], F32, kind="ExternalOutput"
    ).ap()

    with tile.TileContext(nc) as tc, ExitStack() as ctx:
        singles = ctx.enter_context(tc.tile_pool(name="singles", bufs=1))
        encp = ctx.enter_context(tc.tile_pool(name="encp", bufs=12))
        segp = ctx.enter_context(tc.tile_pool(name="segp", bufs=2))
        smalls = ctx.enter_context(tc.tile_pool(name="smalls", bufs=2))
        genp = ctx.enter_context(tc.tile_pool(name="genp", bufs=2))
        ps_seg = ctx.enter_context(tc.tile_pool(name="ps_seg", bufs=2, space="PSUM"))
        ps_tr = ctx.enter_context(tc.tile_pool(name="ps_tr", bufs=1, space="PSUM"))
        ps_pr = ctx.enter_context(tc.tile_pool(name="ps_pr", bufs=2, space="PSUM"))
        ps_fill = ctx.enter_context(tc.tile_pool(name="ps_fill", bufs=1, space="PSUM"))

        # ---- kernel-start staging ----
        # Bulk staging (enc, W) rides the SYNC hardware-DMA ring: the scalar
        # ring moves data at only ~34 GB/s, and a slow staging transfer
        # holds its completion semaphore long enough that an enc trigger
        # round-robined onto the same semaphore stalls the whole stream.
        # Tiny constants ride the gpsimd SWDGE ring instead.
        a_sb = singles.tile([128, BPC, SCHUNKS, N_SENT], FP8)
        ident_sb = singles.tile([N_SENT, N_SENT], F16)
        lens_sb = singles.tile([N_SENT, BPC], F32)
        w_sb = singles.tile([128, DCH, D_OUT], F16)
        sgrid_sb = singles.tile([128, SCHUNKS], F32)
        se_bc = singles.tile([128, BPC, 2 * N_SENT], F32)
        ets0 = [
            encp.tile([128, SS_PER_DMA, D_IN], FP8, tag="enc", name=f"et0_{i}")
            for i in range(N_DMA)
        ]
        for kk in range(N_DMA):
            nc.sync.dma_start(
                out=ets0[kk], in_=enc[0, kk].rearrange("p (t d) -> p t d", d=D_IN)
            )
        nc.sync.dma_start(out=w_sb, in_=wt.rearrange("p (j o) -> p j o", o=D_OUT))
        # A-generation inputs first on the gpsimd ring (needed by ~8 us)
        nc.gpsimd.dma_start(out=sgrid_sb, in_=sgrid)
        for bi in range(BPC):
            row = sten[bi]
            nc.gpsimd.dma_start(
                out=se_bc[:, bi, :],
                in_=bass.AP(
                    tensor=row.tensor, offset=row.offset,
                    ap=[[0, 128], [1, 2 * N_SENT]],
                ),
            )
        nc.gpsimd.dma_start(out=ident_sb, in_=ident)
        nc.gpsimd.dma_start(out=lens_sb, in_=lens)
        # b broadcast to [N_SENT, D_OUT] via stride-0 partition AP (SWDGE)
        b_bc = singles.tile([N_SENT, D_OUT], F32)
        bias_bcast = bass.AP(
            tensor=bias.tensor, offset=bias.offset,
            ap=[[0, N_SENT], [1, D_OUT]],
        )
        nc.gpsimd.dma_start(out=b_bc, in_=bias_bcast)

        # On-device A generation: A[p, k, n] = (s >= start_n) - (s >= end_n)
        # with s = 128k + p, computed on DVE in fp8 straight into a_sb.
        KGEN = SCHUNKS // 4          # k-chunks per gen piece

        def gen_a(bi, ci):
            k0, k1 = ci * KGEN, (ci + 1) * KGEN
            sg = sgrid_sb[:, k0:k1]
            sg3 = bass.AP(
                tensor=sg.tensor, offset=sg.offset,
                ap=[list(sg.ap[0]), list(sg.ap[1]), [0, N_SENT]],
            )
            stb = se_bc[:, bi, 0:N_SENT]
            st3 = bass.AP(
                tensor=stb.tensor, offset=stb.offset,
                ap=[list(stb.ap[0]), [0, KGEN], list(stb.ap[1])],
            )
            enb = se_bc[:, bi, N_SENT : 2 * N_SENT]
            en3 = bass.AP(
                tensor=enb.tensor, offset=enb.offset,
                ap=[list(enb.ap[0]), [0, KGEN], list(enb.ap[1])],
            )
            t0 = genp.tile([128, KGEN, N_SENT], F32, tag="ge_s", name="t0")
            t1 = genp.tile([128, KGEN, N_SENT], F32, tag="ge_e", name="t1")
            nc.vector.tensor_tensor(out=t0, in0=sg3, in1=st3, op=mybir.AluOpType.is_ge)
            nc.vector.tensor_tensor(out=t1, in0=sg3, in1=en3, op=mybir.AluOpType.is_ge)
            nc.vector.tensor_tensor(
                out=a_sb[:, bi, k0:k1, :], in0=t0, in1=t1,
                op=mybir.AluOpType.subtract,
            )

        # ACT table preload: dummy Copy+Exp at the start.  Copy and Exp live
        # in the SAME activation table (Ln, which lives in another and would
        # thrash the slot every batch, is computed manually on DVE), so the
        # table loads exactly once and no 1.3 us table load ever lands on a
        # batch tail's critical path.
        dummy = singles.tile([1, 4], F32)
        nc.gpsimd.memset(dummy, 1.0)
        nc.scalar.copy(out=dummy[:, 3:4], in_=dummy[:, 1:2])
        nc.scalar.activation(
            out=dummy[:, 2:3], in_=dummy[:, 0:1],
            func=mybir.ActivationFunctionType.Exp,
        )

        # HAM warm-up: dense dummy DR matmuls on zeroed scratch while the
        # first enc slab is still in flight.  ~16 x 512 cycles of gap-free
        # PE busy trips the Activity_SHORT window and lifts the PE clock
        # from 1.2 GHz to 2.4 GHz before real work arrives.
        #
        # All fillers accumulate into ONE psum tile as a single open
        # accumulation group: per-filler pool tiles would be recycled via
        # semaphores, serializing the PE at ~1.2 us per filler pair and
        # destroying the very density the fillers exist to provide.
        scr_w = singles.tile([128, 2, N_SENT], FP8)
        scr_x = singles.tile([128, 2, 512], FP8)
        nc.vector.memset(scr_w, 0.0)
        nc.vector.memset(scr_x, 0.0)
        # -1.0 bias vector for the Newton-step exp (no registered const AP)
        neg1 = singles.tile([N_SENT, 1], F32)
        nc.vector.memset(neg1, -1.0)
        fps = ps_fill.tile([N_SENT, 512], F32, tag="fill")
        n_fillers_total = WARMUP_MMS + BPC * N_DMA * FILLERS_PER_SLAB
        fill_count = [0]

        def filler(n):
            for _ in range(n):
                i = fill_count[0]
                fill_count[0] += 1
                nc.tensor.matmul(
                    fps, lhsT=scr_w, rhs=scr_x,
                    start=(i == 0), stop=(i == n_fillers_total - 1),
                    perf_mode=mybir.MatmulPerfMode.DoubleRow,
                )

        # batch 0's A right away (chunked so the first matmuls aren't gated
        # on the full batch), then the warm-up burst
        for ci in range(4):
            gen_a(0, ci)
        filler(WARMUP_MMS)

        n_pairs = SCHUNKS // 2
        psums = {}

        def tail_pieces(bi):
            """Per-batch tail, split into pieces that interleave with the
            next batch's seg matmuls.  All tensor-engine tail work is fp16:
            transposes use the fast transpose_mode path and the projection
            weight loads are FWL-overlapped, so the tail costs the PE
            ~2.5 us per batch instead of ~4.5 us in fp32."""
            st = {}

            def p_evict():
                ps0, ps1 = psums.pop(bi)
                seg = segp.tile([N_SENT, D_IN], F16, tag="seg", name="seg")
                # halves evicted on DVE and ACT in parallel (Copy shares the
                # resident Exp table, so this costs no table load)
                nc.vector.tensor_copy(out=seg[:, 0:512], in_=ps0)
                nc.scalar.copy(out=seg[:, 512:1024], in_=ps1)
                st["seg"] = seg

            def p_transpose():
                pt = ps_tr.tile([128, DCH, N_SENT], F16, tag="pt")
                for j in range(DCH):
                    nc.tensor.transpose(
                        out=pt[:, j, :],
                        in_=st["seg"][:, j * 128 : (j + 1) * 128],
                        identity=ident_sb,
                    )
                segt = segp.tile([128, DCH, N_SENT], F16, tag="segT", name="segt")
                nc.vector.tensor_copy(out=segt, in_=pt)
                st["segt"] = segt

            def p_project():
                pp = ps_pr.tile([N_SENT, D_OUT], F32, tag="pp", name="pp")
                for j in range(DCH):
                    nc.tensor.matmul(
                        pp,
                        lhsT=st["segt"][:, j, :],
                        rhs=w_sb[:, j, :],
                        start=(j == 0),
                        stop=(j == DCH - 1),
                    )
                st["pp"] = pp

            def p_soft():
                # sv = pp + len * b
                sv = smalls.tile([N_SENT, D_OUT], F32, tag="sv")
                nc.vector.scalar_tensor_tensor(
                    out=sv,
                    in0=b_bc,
                    scalar=lens_sb[:, bi : bi + 1],
                    in1=st["pp"],
                    op0=mybir.AluOpType.mult,
                    op1=mybir.AluOpType.add,
                )
                negmax = smalls.tile([N_SENT, 1], F32, tag="negmax")
                nc.vector.tensor_reduce(
                    out=negmax, in_=sv, axis=mybir.AxisListType.X,
                    op=mybir.AluOpType.max, negate=True,
                )
                # ex = exp(sv - max) with the sum fused in as accum_out
                ex = smalls.tile([N_SENT, D_OUT], F32, tag="ex")
                ssum = smalls.tile([N_SENT, 1], F32, tag="ssum")
                nc.scalar.activation(
                    out=ex, in_=sv,
                    func=mybir.ActivationFunctionType.Exp,
                    bias=negmax, accum_out=ssum,
                )
                # lse = ln(ssum), ssum in [1, 256], computed WITHOUT the ACT
                # Ln table (which would evict the Exp table and cost a 1.3 us
                # reload per batch):  exponent-trick estimate
                #   y0 = ln2 * (bits(x)/2^23 - 126.94269504)   (|err| < .043)
                # then one Newton step  y1 = y0 - 1 + x*exp(-y0)  using the
                # resident Exp table (|err| ~ 1e-3).
                uf = smalls.tile([N_SENT, 1], F32, tag="uf")
                nc.vector.tensor_copy(out=uf, in_=ssum.bitcast(mybir.dt.int32))
                LN2 = 0.6931471805599453
                y0m1 = smalls.tile([N_SENT, 1], F32, tag="y0m1")
                nc.vector.tensor_scalar(
                    out=y0m1, in0=uf,
                    scalar1=LN2 / (1 << 23),
                    scalar2=126.94269504 * LN2 + 1.0,
                    op0=mybir.AluOpType.mult, op1=mybir.AluOpType.subtract,
                )
                texp = smalls.tile([N_SENT, 1], F32, tag="texp")
                nc.scalar.activation(
                    out=texp, in_=y0m1,
                    func=mybir.ActivationFunctionType.Exp,
                    scale=-1.0, bias=neg1,
                )
                lse = smalls.tile([N_SENT, 1], F32, tag="lse")
                nc.vector.scalar_tensor_tensor(
                    out=lse, in0=ssum, scalar=texp, in1=y0m1,
                    op0=mybir.AluOpType.mult, op1=mybir.AluOpType.add,
                )
                # out = (sv + negmax) - lse
                ot = smalls.tile([N_SENT, D_OUT], F32, tag="ot")
                nc.vector.tensor_scalar(
                    out=ot, in0=sv, scalar1=negmax, scalar2=lse,
                    op0=mybir.AluOpType.add, op1=mybir.AluOpType.subtract,
                )
                # out trigger on gpsimd too: on the sync ring it would sit
                # ahead of later enc triggers and stall the enc stream until
                # this batch's softmax completes.
                nc.gpsimd.dma_start(out=out[bi], in_=ot)

            return [p_evict, p_transpose, p_project, p_soft]

        pending = []

        for bi in range(BPC):
            if bi > 0:
                pending.extend(tail_pieces(bi - 1))
            ps0 = ps_seg.tile([N_SENT, 512], F32, tag="ps0")
            ps1 = ps_seg.tile([N_SENT, 512], F32, tag="ps1")
            psums[bi] = (ps0, ps1)
            for kk in range(N_DMA):
                if bi == 0:
                    et = ets0[kk]
                else:
                    et = encp.tile([128, SS_PER_DMA, D_IN], FP8, tag="enc")
                    nc.sync.dma_start(
                        out=et,
                        in_=enc[bi, kk].rearrange("p (t d) -> p t d", d=D_IN),
                    )
                if pending:
                    pending.pop(0)()
                # next batch's A, one chunk per slab, behind the tail piece
                # on the DVE queue so tails aren't delayed
                if bi + 1 < BPC:
                    gen_a(bi + 1, kk)
                for u in range(SS_PER_DMA // 2):
                    pair = (kk * SS_PER_DMA) // 2 + u
                    lhsT = a_sb[:, bi, 2 * pair : 2 * pair + 2, :]
                    for dh in range(2):
                        rhs = et[:, 2 * u : 2 * u + 2, dh * 512 : (dh + 1) * 512]
                        nc.tensor.matmul(
                            ps0 if dh == 0 else ps1,
                            lhsT=lhsT,
                            rhs=rhs,
                            start=(pair == 0),
                            stop=(pair == n_pairs - 1),
                            perf_mode=mybir.MatmulPerfMode.DoubleRow,
                        )
                filler(FILLERS_PER_SLAB)
        for piece in pending:
            piece()
        for piece in tail_pieces(BPC - 1):
            piece()

    nc.compile()
    return nc


_PROGRAM = None


def _get_program():
    global _PROGRAM
    if _PROGRAM is None:
        _PROGRAM = _build_program()
    return _PROGRAM


def _quantize_diffuse(enc):
    """fp8 e4m3 quantization with error diffusion along s (block=128).

    Within each contiguous 128-position block the rounding error of each
    element is carried into the next, so any in-block partial sum of the
    quantized values equals the exact partial sum plus at most ~one
    rounding step.  Segment sums then see only ~one step of error per
    block boundary crossed instead of sqrt(len) growth.
    """
    enc_r = enc.reshape(B, SCHUNKS, 128, D_IN)
    q = np.empty((B, SCHUNKS, 128, D_IN), dtype=_E4NP)
    carry = np.zeros((B, SCHUNKS, D_IN), dtype=np.float32)
    for i in range(128):
        t = enc_r[:, :, i, :] + carry
        qi = t.astype(_E4NP)
        q[:, :, i, :] = qi
        carry = t - qi.astype(np.float32)
    return q  # [B, k, p, D] with s = k*128 + p


def _host_prep(enc_output, W, b, cls_pos, last_sep):
    enc = np.asarray(enc_output, dtype=np.float32)
    q = _quantize_diffuse(enc)
    # [B, k, p, D] -> [B, N_DMA, 128(p), SS_PER_DMA(t) * D]  with k = kk*SS+t
    enc8 = np.ascontiguousarray(
        q.reshape(B, N_DMA, SS_PER_DMA, 128, D_IN)
        .transpose(0, 1, 3, 2, 4)
        .reshape(B, N_DMA, 128, SS_PER_DMA * D_IN)
    )
    wf = np.asarray(W, dtype=np.float32)
    # [D_IN, D_OUT] -> fp16 [128(p), DCH(j) * D_OUT] with d = j*128+p
    wf = np.ascontiguousarray(
        wf.reshape(DCH, 128, D_OUT).transpose(1, 0, 2).reshape(128, DCH * D_OUT)
    ).astype(_F16NP)
    bf = np.ascontiguousarray(np.asarray(b, dtype=np.float32))
    starts = np.asarray(cls_pos).astype(np.int64)                    # [B, N]
    lsep = np.asarray(last_sep).astype(np.int64)                     # [B]
    ends = np.concatenate([starts[:, 1:], (lsep + 1)[:, None]], axis=1)
    # torch semantics for the last segment: if end <= start, sum to seq end
    ends[:, -1] = np.where(ends[:, -1] > starts[:, -1], ends[:, -1], S)
    lens = (ends - starts).astype(np.float32)                        # [B, N]

    # per-batch [starts | ends] rows for on-device A generation (f32-exact)
    sten = np.concatenate(
        [starts.astype(np.float32), ends.astype(np.float32)], axis=1
    )                                                                # [B, 2N]
    return enc8, wf, bf, sten, lens


def kernel(enc_output, W, b, max_num_sent, cls_pos, last_sep, _trace=False):
    enc8, wf, bf, sten, lens = _host_prep(enc_output, W, b, cls_pos, last_sep)
    ident = np.eye(N_SENT, dtype=_F16NP)
    # s-grid [128(p), SCHUNKS(k)] with s = 128k + p
    sgrid = (
        np.arange(128, dtype=np.float32)[:, None]
        + 128.0 * np.arange(SCHUNKS, dtype=np.float32)[None, :]
    )
    sgrid = np.ascontiguousarray(sgrid)

    nc = _get_program()
    in_maps = []
    for c in range(N_CORES):
        bsl = slice(c * BPC, (c + 1) * BPC)
        in_maps.append(
            {
                "enc": enc8[bsl],
                "w": wf,
                "bias": bf,
                "sgrid": sgrid,
                "sten": np.ascontiguousarray(sten[bsl]),
                "lens": np.ascontiguousarray(lens[bsl].T),
                "ident": ident,
            }
        )
    res = bass_utils.run_bass_kernel_spmd(
        nc, in_maps, core_ids=list(range(N_CORES)), trace=_trace
    )
    out = np.concatenate(
        [res.results[c]["out"][None] for c in range(N_CORES)], axis=0
    ).reshape(B, N_SENT, D_OUT)
    if _trace:
        kernel._last_result = res
    return out.astype(np.float32)


# revision 33
# speedup vs baseline: 1.2696x; 1.2696x over previous
"""Trainium2 Bass kernel for nn_DialogActLabeller (segment_reduce).

Computes, for input enc_output [32, 4096, 1024], W [1024, 256], b [256],
cls_pos [32, 64], last_sep [32]:

    x = enc_output @ W + b                      # [B, S, 256]
    seg[b, n] = sum_{s in [start_n, end_n)} x[b, s, :]
    out = log_softmax(seg, axis=-1)             # [B, 64, 256]

Key algebraic restructure: the projection is linear, so segment-reduce
FIRST on enc_output (via a matmul with a 0/1 segment-indicator matrix A),
then project the tiny [64, 1024] per-batch result with W, and add
len_n * b for the bias.  This reads enc_output exactly once from HBM and
does ~1/32 of the naive FLOPs.

The kernel is HBM-bound (enc_output is 512 MiB), so enc is shipped as
fp8 e4m3 with error diffusion along s so segment sums telescope (see
_quantize_diffuse).  The segment-reduce matmul runs in fp8 DoubleRow
mode; in the warm (K=8/8) HAM state the PE issues one F=512 DR matmul
every 213 ns with the weight loads fully overlapped, which keeps pace
with the 16-engine enc DMA stream at ~358 GB/s.

PE clock management: the PE_HAM clock gate defaults to K=4/8 (1.2 GHz)
and only reaches 2.4 GHz after ~3.4 us of gap-free busy.  A warm-up
burst of dummy DR matmuls runs during the DMA ramp so the array is
already warm when the first enc slab lands, and small filler matmuls
are sprinkled between slabs so no idle window re-throttles the clock.

The per-batch tail (PSUM evict, transpose, projection, softmax) runs in
fp16 (fast transposes + FWL-overlapped weight loads, ample precision
against the 2e-2 gate), is software-pipelined one batch behind the enc
stream, and the softmax chain is fused:  exp(x-max) with the max as the
ACT bias input and the sum as ACT accum_out, then a single
(sv + negmax) - lse vector op.  Both ACT tables (Copy/Exp, Ln) are
preloaded at kernel start so no table load lands on the critical path.
Outputs DMA per batch as soon as they are ready.

Sharding: pure data parallel, 4 batch rows per core across 8 cores
(W, b replicated), no cross-core communication.
"""

import numpy as np

import concourse.bacc as bacc
import concourse.bass as bass
import concourse.tile as tile
from concourse import mybir
from concourse import bass_utils
from contextlib import ExitStack

# Problem shapes (hardcoded per contract)
B, S, D_IN, D_OUT, N_SENT = 32, 4096, 1024, 256, 64
N_CORES = 8
BPC = B // N_CORES          # batches per core
SCHUNKS = S // 128          # 32 sequence chunks of 128
DCH = D_IN // 128           # 8 d_in chunks of 128
SS_PER_DMA = 8              # s-chunks per enc DMA (1 MiB fp8 transfers)
N_DMA = SCHUNKS // SS_PER_DMA

F32 = mybir.dt.float32
F16 = mybir.dt.float16
FP8 = mybir.dt.float8e4
_E4NP = mybir.dt.np(FP8)    # ml_dtypes.float8_e4m3
_F16NP = mybir.dt.np(F16)

WARMUP_MMS = 12             # dummy DR matmuls to un-throttle the PE HAM
FILLERS_PER_SLAB = 1        # keep-warm matmuls after each slab's real MMs


def _build_program():
    nc = bacc.Bacc("TRN2", debug=False)

    enc = nc.dram_tensor(
        "enc", [BPC, N_DMA, 128, SS_PER_DMA * D_IN], FP8, kind="ExternalInput"
    ).ap()
    # W host-pre-tiled to fp16 [128, j, o] with d = j*128 + p
    wt = nc.dram_tensor("w", [128, DCH * D_OUT], F16, kind="ExternalInput").ap()
    bias = nc.dram_tensor("bias", [D_OUT], F32, kind="ExternalInput").ap()
    # segment-indicator matrices in fp8, host-pre-tiled to the exact SBUF
    # layout [128(p), BPC, SCHUNKS, N_SENT] so the DMA is fully contiguous
    amat = nc.dram_tensor(
        "amat", [128, BPC * SCHUNKS * N_SENT], FP8, kind="ExternalInput"
    ).ap()
    lens = nc.dram_tensor("lens", [N_SENT, BPC], F32, kind="ExternalInput").ap()
    ident = nc.dram_tensor("ident", [N_SENT, N_SENT], F16, kind="ExternalInput").ap()
    out = nc.dram_tensor(
        "out", [BPC, N_SENT, D_OUT], F32, kind="ExternalOutput"
    ).ap()

    with tile.TileContext(nc) as tc, ExitStack() as ctx:
        singles = ctx.enter_context(tc.tile_pool(name="singles", bufs=1))
        encp = ctx.enter_context(tc.tile_pool(name="encp", bufs=12))
        segp = ctx.enter_context(tc.tile_pool(name="segp", bufs=2))
        smalls = ctx.enter_context(tc.tile_pool(name="smalls", bufs=2))
        ps_seg = ctx.enter_context(tc.tile_pool(name="ps_seg", bufs=2, space="PSUM"))
        ps_tr = ctx.enter_context(tc.tile_pool(name="ps_tr", bufs=1, space="PSUM"))
        ps_pr = ctx.enter_context(tc.tile_pool(name="ps_pr", bufs=2, space="PSUM"))
        ps_fill = ctx.enter_context(tc.tile_pool(name="ps_fill", bufs=1, space="PSUM"))

        # ---- kernel-start staging ----
        # enc, W and batch 0's A slab ride the FAST sync hardware-DMA ring.
        # The remaining A slabs ride the slow (~34 GB/s) scalar ring, which
        # yields to the sync ring: their ~0.8 MB stops competing with the
        # enc stream on the DMA engines' critical finish, and they still
        # land many microseconds before their batches need them.  Emission
        # order is chosen so the semaphore round-robin never makes a sync
        # trigger wait on a slow scalar-ring transfer that is still in
        # flight when the trigger issues.
        amat_v = amat.rearrange("p (b k n) -> p b k n", k=SCHUNKS, n=N_SENT)
        a_sb = singles.tile([128, BPC, SCHUNKS, N_SENT], FP8)
        ident_sb = singles.tile([N_SENT, N_SENT], F16)
        lens_sb = singles.tile([N_SENT, BPC], F32)
        w_sb = singles.tile([128, DCH, D_OUT], F16)
        ets0 = [
            encp.tile([128, SS_PER_DMA, D_IN], FP8, tag="enc", name=f"et0_{i}")
            for i in range(N_DMA)
        ]
        nc.sync.dma_start(out=ets0[0], in_=enc[0, 0].rearrange("p (t d) -> p t d", d=D_IN))
        nc.sync.dma_start(out=ets0[1], in_=enc[0, 1].rearrange("p (t d) -> p t d", d=D_IN))
        nc.sync.dma_start(out=a_sb[:, 0:1], in_=amat_v[:, 0:1])
        nc.sync.dma_start(out=ets0[2], in_=enc[0, 2].rearrange("p (t d) -> p t d", d=D_IN))
        nc.sync.dma_start(out=ets0[3], in_=enc[0, 3].rearrange("p (t d) -> p t d", d=D_IN))
        nc.sync.dma_start(out=w_sb, in_=wt.rearrange("p (j o) -> p j o", o=D_OUT))
        # scavenger ring: later batches' A slabs
        for bi in range(1, BPC):
            nc.scalar.dma_start(out=a_sb[:, bi : bi + 1], in_=amat_v[:, bi : bi + 1])
        # tiny constants on the gpsimd SWDGE ring
        nc.gpsimd.dma_start(out=ident_sb, in_=ident)
        nc.gpsimd.dma_start(out=lens_sb, in_=lens)
        # b broadcast to [N_SENT, D_OUT] via stride-0 partition AP (SWDGE)
        b_bc = singles.tile([N_SENT, D_OUT], F32)
        bias_bcast = bass.AP(
            tensor=bias.tensor, offset=bias.offset,
            ap=[[0, N_SENT], [1, D_OUT]],
        )
        nc.gpsimd.dma_start(out=b_bc, in_=bias_bcast)

        # ACT table preload: dummy Copy+Exp at the start.  Copy and Exp live
        # in the SAME activation table (Ln, which lives in another and would
        # thrash the slot every batch, is computed manually on DVE), so the
        # table loads exactly once and no 1.3 us table load ever lands on a
        # batch tail's critical path.
        dummy = singles.tile([1, 4], F32)
        nc.gpsimd.memset(dummy, 1.0)
        nc.scalar.copy(out=dummy[:, 3:4], in_=dummy[:, 1:2])
        nc.scalar.activation(
            out=dummy[:, 2:3], in_=dummy[:, 0:1],
            func=mybir.ActivationFunctionType.Exp,
        )

        # HAM warm-up: dense dummy DR matmuls on zeroed scratch while the
        # first enc slab is still in flight.  ~16 x 512 cycles of gap-free
        # PE busy trips the Activity_SHORT window and lifts the PE clock
        # from 1.2 GHz to 2.4 GHz before real work arrives.
        #
        # All fillers accumulate into ONE psum tile as a single open
        # accumulation group: per-filler pool tiles would be recycled via
        # semaphores, serializing the PE at ~1.2 us per filler pair and
        # destroying the very density the fillers exist to provide.
        scr_w = singles.tile([128, 2, N_SENT], FP8)
        scr_x = singles.tile([128, 2, 512], FP8)
        nc.vector.memset(scr_w, 0.0)
        nc.vector.memset(scr_x, 0.0)
        # -1.0 bias vector for the Newton-step exp (no registered const AP)
        neg1 = singles.tile([N_SENT, 1], F32)
        nc.vector.memset(neg1, -1.0)
        fps = ps_fill.tile([N_SENT, 512], F32, tag="fill")
        n_fillers_total = WARMUP_MMS + BPC * N_DMA * FILLERS_PER_SLAB
        fill_count = [0]

        def filler(n):
            for _ in range(n):
                i = fill_count[0]
                fill_count[0] += 1
                nc.tensor.matmul(
                    fps, lhsT=scr_w, rhs=scr_x,
                    start=(i == 0), stop=(i == n_fillers_total - 1),
                    perf_mode=mybir.MatmulPerfMode.DoubleRow,
                )

        filler(WARMUP_MMS)

        n_pairs = SCHUNKS // 2
        psums = {}

        def tail_pieces(bi):
            """Per-batch tail, split into pieces that interleave with the
            next batch's seg matmuls.  All tensor-engine tail work is fp16:
            transposes use the fast transpose_mode path and the projection
            weight loads are FWL-overlapped, so the tail costs the PE
            ~2.5 us per batch instead of ~4.5 us in fp32."""
            st = {}

            def p_evict():
                ps0, ps1 = psums.pop(bi)
                seg = segp.tile([N_SENT, D_IN], F16, tag="seg", name="seg")
                # halves evicted on DVE and ACT in parallel (Copy shares the
                # resident Exp table, so this costs no table load)
                nc.vector.tensor_copy(out=seg[:, 0:512], in_=ps0)
                nc.scalar.copy(out=seg[:, 512:1024], in_=ps1)
                st["seg"] = seg

            def p_transpose():
                pt = ps_tr.tile([128, DCH, N_SENT], F16, tag="pt")
                for j in range(DCH):
                    nc.tensor.transpose(
                        out=pt[:, j, :],
                        in_=st["seg"][:, j * 128 : (j + 1) * 128],
                        identity=ident_sb,
                    )
                segt = segp.tile([128, DCH, N_SENT], F16, tag="segT", name="segt")
                nc.vector.tensor_copy(out=segt, in_=pt)
                st["segt"] = segt

            def p_project():
                pp = ps_pr.tile([N_SENT, D_OUT], F32, tag="pp", name="pp")
                for j in range(DCH):
                    nc.tensor.matmul(
                        pp,
                        lhsT=st["segt"][:, j, :],
                        rhs=w_sb[:, j, :],
                        start=(j == 0),
                        stop=(j == DCH - 1),
                    )
                st["pp"] = pp

            def p_soft():
                # sv = pp + len * b
                sv = smalls.tile([N_SENT, D_OUT], F32, tag="sv")
                nc.vector.scalar_tensor_tensor(
                    out=sv,
                    in0=b_bc,
                    scalar=lens_sb[:, bi : bi + 1],
                    in1=st["pp"],
                    op0=mybir.AluOpType.mult,
                    op1=mybir.AluOpType.add,
                )
                negmax = smalls.tile([N_SENT, 1], F32, tag="negmax")
                nc.vector.tensor_reduce(
                    out=negmax, in_=sv, axis=mybir.AxisListType.X,
                    op=mybir.AluOpType.max, negate=True,
                )
                # ex = exp(sv - max) with the sum fused in as accum_out
                ex = smalls.tile([N_SENT, D_OUT], F32, tag="ex")
                ssum = smalls.tile([N_SENT, 1], F32, tag="ssum")
                nc.scalar.activation(
                    out=ex, in_=sv,
                    func=mybir.ActivationFunctionType.Exp,
                    bias=negmax, accum_out=ssum,
                )
                # lse = ln(ssum), ssum in [1, 256], computed WITHOUT the ACT
                # Ln table (which would evict the Exp table and cost a 1.3 us
                # reload per batch):  exponent-trick estimate
                #   y0 = ln2 * (bits(x)/2^23 - 126.94269504)   (|err| < .043)
                # then one Newton step  y1 = y0 - 1 + x*exp(-y0)  using the
                # resident Exp table (|err| ~ 1e-3).
                uf = smalls.tile([N_SENT, 1], F32, tag="uf")
                nc.vector.tensor_copy(out=uf, in_=ssum.bitcast(mybir.dt.int32))
                LN2 = 0.6931471805599453
                y0m1 = smalls.tile([N_SENT, 1], F32, tag="y0m1")
                nc.vector.tensor_scalar(
                    out=y0m1, in0=uf,
                    scalar1=LN2 / (1 << 23),
                    scalar2=126.94269504 * LN2 + 1.0,
                    op0=mybir.AluOpType.mult, op1=mybir.AluOpType.subtract,
                )
                texp = smalls.tile([N_SENT, 1], F32, tag="texp")
                nc.scalar.activation(
                    out=texp, in_=y0m1,
                    func=mybir.ActivationFunctionType.Exp,
                    scale=-1.0, bias=neg1,
                )
                lse = smalls.tile([N_SENT, 1], F32, tag="lse")
                nc.vector.scalar_tensor_tensor(
                    out=lse, in0=ssum, scalar=texp, in1=y0m1,
                    op0=mybir.AluOpType.mult, op1=mybir.AluOpType.add,
                )
                # out = (sv + negmax) - lse
                ot = smalls.tile([N_SENT, D_OUT], F32, tag="ot")
                nc.vector.tensor_scalar(
                    out=ot, in0=sv, scalar1=negmax, scalar2=lse,
                    op0=mybir.AluOpType.add, op1=mybir.AluOpType.subtract,
                )
                # out trigger on gpsimd too: on the sync ring it would sit
                # ahead of later enc triggers and stall the enc stream until
                # this batch's softmax completes.
                nc.gpsimd.dma_start(out=out[bi], in_=ot)

            return [p_evict, p_transpose, p_project, p_soft]

        pending = []

        for bi in range(BPC):
            if bi > 0:
                pending.extend(tail_pieces(bi - 1))
            ps0 = ps_seg.tile([N_SENT, 512], F32, tag="ps0")
            ps1 = ps_seg.tile([N_SENT, 512], F32, tag="ps1")
            psums[bi] = (ps0, ps1)
            for kk in range(N_DMA):
                if bi == 0:
                    et = ets0[kk]
                else:
                    et = encp.tile([128, SS_PER_DMA, D_IN], FP8, tag="enc")
                    nc.sync.dma_start(
                        out=et,
                        in_=enc[bi, kk].rearrange("p (t d) -> p t d", d=D_IN),
                    )
                if pending:
                    pending.pop(0)()
                for u in range(SS_PER_DMA // 2):
                    pair = (kk * SS_PER_DMA) // 2 + u
                    lhsT = a_sb[:, bi, 2 * pair : 2 * pair + 2, :]
                    for dh in range(2):
                        rhs = et[:, 2 * u : 2 * u + 2, dh * 512 : (dh + 1) * 512]
                        nc.tensor.matmul(
                            ps0 if dh == 0 else ps1,
                            lhsT=lhsT,
                            rhs=rhs,
                            start=(pair == 0),
                            stop=(pair == n_pairs - 1),
                            perf_mode=mybir.MatmulPerfMode.DoubleRow,
                        )
                filler(FILLERS_PER_SLAB)
        for piece in pending:
            piece()
        for piece in tail_pieces(BPC - 1):
            piece()

    nc.compile()
    return nc


_PROGRAM = None


def _get_program():
    global _PROGRAM
    if _PROGRAM is None:
        _PROGRAM = _build_program()
    return _PROGRAM


def _quantize_diffuse(enc):
    """fp8 e4m3 quantization with error diffusion along s (block=128).

    Within each contiguous 128-position block the rounding error of each
    element is carried into the next, so any in-block partial sum of the
    quantized values equals the exact partial sum plus at most ~one
    rounding step.  Segment sums then see only ~one step of error per
    block boundary crossed instead of sqrt(len) growth.
    """
    enc_r = enc.reshape(B, SCHUNKS, 128, D_IN)
    q = np.empty((B, SCHUNKS, 128, D_IN), dtype=_E4NP)
    carry = np.zeros((B, SCHUNKS, D_IN), dtype=np.float32)
    for i in range(128):
        t = enc_r[:, :, i, :] + carry
        qi = t.astype(_E4NP)
        q[:, :, i, :] = qi
        carry = t - qi.astype(np.float32)
    return q  # [B, k, p, D] with s = k*128 + p


def _host_prep(enc_output, W, b, cls_pos, last_sep):
    enc = np.asarray(enc_output, dtype=np.float32)
    q = _quantize_diffuse(enc)
    # [B, k, p, D] -> [B, N_DMA, 128(p), SS_PER_DMA(t) * D]  with k = kk*SS+t
    enc8 = np.ascontiguousarray(
        q.reshape(B, N_DMA, SS_PER_DMA, 128, D_IN)
        .transpose(0, 1, 3, 2, 4)
        .reshape(B, N_DMA, 128, SS_PER_DMA * D_IN)
    )
    wf = np.asarray(W, dtype=np.float32)
    # [D_IN, D_OUT] -> fp16 [128(p), DCH(j) * D_OUT] with d = j*128+p
    wf = np.ascontiguousarray(
        wf.reshape(DCH, 128, D_OUT).transpose(1, 0, 2).reshape(128, DCH * D_OUT)
    ).astype(_F16NP)
    bf = np.ascontiguousarray(np.asarray(b, dtype=np.float32))
    starts = np.asarray(cls_pos).astype(np.int64)                    # [B, N]
    lsep = np.asarray(last_sep).astype(np.int64)                     # [B]
    ends = np.concatenate([starts[:, 1:], (lsep + 1)[:, None]], axis=1)
    # torch semantics for the last segment: if end <= start, sum to seq end
    ends[:, -1] = np.where(ends[:, -1] > starts[:, -1], ends[:, -1], S)
    lens = (ends - starts).astype(np.float32)                        # [B, N]

    s = np.arange(S, dtype=np.int64)
    afull = (s[None, :, None] >= starts[:, None, :]) & (
        s[None, :, None] < ends[:, None, :]
    )                                                                # [B, S, N]
    return enc8, wf, bf, afull, lens


def _amat_tile(afull_c):
    """[BPC, S, N] bool -> contiguous [128(p), BPC, SCHUNKS, N] fp8 bytes."""
    a = (
        afull_c.reshape(BPC, SCHUNKS, 128, N_SENT)
        .transpose(2, 0, 1, 3)                       # [128, BPC, SCHUNKS, N]
        .reshape(128, BPC * SCHUNKS * N_SENT)
        .astype(np.float32)
        .astype(_E4NP)                               # 0.0 / 1.0 exact
    )
    return np.ascontiguousarray(a)


def kernel(enc_output, W, b, max_num_sent, cls_pos, last_sep, _trace=False):
    enc8, wf, bf, afull, lens = _host_prep(enc_output, W, b, cls_pos, last_sep)
    ident = np.eye(N_SENT, dtype=_F16NP)

    nc = _get_program()
    in_maps = []
    for c in range(N_CORES):
        bsl = slice(c * BPC, (c + 1) * BPC)
        in_maps.append(
            {
                "enc": enc8[bsl],
                "w": wf,
                "bias": bf,
                "amat": _amat_tile(afull[bsl]),
                "lens": np.ascontiguousarray(lens[bsl].T),
                "ident": ident,
            }
        )
    res = bass_utils.run_bass_kernel_spmd(
        nc, in_maps, core_ids=list(range(N_CORES)), trace=_trace
    )
    out = np.concatenate(
        [res.results[c]["out"][None] for c in range(N_CORES)], axis=0
    ).reshape(B, N_SENT, D_OUT)
    if _trace:
        kernel._last_result = res
    return out.astype(np.float32)
